# revision 6
# baseline (speedup 1.0000x reference)
"""Self-contained Trainium2 Bass kernel for CoherenceAttention.

Problem: out = x + Softmax(mask, (LN(x) Wq^T)(LN(x) Wk^T)^T / sqrt(D)) (LN(x) Wv^T) Wo^T
Shapes: x (4, 2048, 768), weights (768, 768), LN affine (768,).

Sharding: 8 cores = (batch, query-half). Each core receives its batch's x with
its own 1024 query rows first (attention is permutation-invariant over keys),
computes LN + K/V for all 2048 keys (duplicated within the batch pair; no
collectives), and Q/scores/softmax/output projection for its 1024 queries.

Device kernel layout strategy (per core):
  P1: token-major LN (bn_stats/bn_aggr) then PE-transpose -> yT [d, s] (feature-major)
  P2: QT = (Wq' yT)  [e, q] ; KT [e, k] ; V token-major [k, e] spilled to DRAM
  P3: per 512-query chunk: scoresT[k,q] in PSUM -> exp on ACT -> SBUF;
      attn_outT[e,q] += V-block^T-free matmuls over k; denominator via
      elementwise-accumulate + ones-matmul partition reduce (broadcast form);
      normalize; out[q,f] = attn_norm^T-matmuls with Wo^T + residual.
All matmuls run as float32r (full fp32 data, 1 cycle/row on the PE).
"""

import numpy as np

B, S, D = 4, 2048, 768
N_CORES = 8
P = 128
SQ = S // 2           # queries per core
DT = D // P           # 6 contraction tiles
ST = S // P           # 16 token tiles
KB = S // P           # 16 key blocks
QC = 512              # query chunk (PSUM bank width in fp32)
NCH = SQ // QC        # 2 chunks per core
LN_EPS = 1e-5

_BUILD_CACHE = {}


def _build(has_bias: bool, use_mask: bool, use_f32r: bool):
    import concourse.bacc as bacc
    import concourse.mybir as mybir
    import concourse.tile as tile
    from concourse.masks import make_identity
    from contextlib import ExitStack

    f32 = mybir.dt.float32
    f32r = mybir.dt.float32r if use_f32r else f32

    def mm(ap):
        return ap

    nc = bacc.Bacc("TRN2", target_bir_lowering=False, debug=False,
                   num_devices=N_CORES)

    x = nc.dram_tensor("x", [S, D], f32, kind="ExternalInput")
    wqt = nc.dram_tensor("wqt", [D, D], f32r, kind="ExternalInput")
    wkt = nc.dram_tensor("wkt", [D, D], f32r, kind="ExternalInput")
    wvt = nc.dram_tensor("wvt", [D, D], f32r, kind="ExternalInput")
    wot = nc.dram_tensor("wot", [D, D], f32r, kind="ExternalInput")
    if has_bias:
        cq = nc.dram_tensor("cq", [1, D], f32r, kind="ExternalInput")
        ck = nc.dram_tensor("ck", [1, D], f32r, kind="ExternalInput")
        cv = nc.dram_tensor("cv", [1, D], f32r, kind="ExternalInput")
    if use_mask:
        amask = nc.dram_tensor("amask", [S, SQ], f32, kind="ExternalInput")
    out_d = nc.dram_tensor("out", [SQ, D], f32, kind="ExternalOutput")

    sub = mybir.AluOpType.subtract
    mult = mybir.AluOpType.mult
    Exp = mybir.ActivationFunctionType.Exp
    Sqrt = mybir.ActivationFunctionType.Sqrt

    with tile.TileContext(nc) as tc, ExitStack() as outer:
        const = outer.enter_context(tc.tile_pool(name="const", bufs=1))
        dram = outer.enter_context(tc.tile_pool(name="dram", bufs=1, space="DRAM"))
        wo_pool = outer.enter_context(tc.tile_pool(name="wop", bufs=1))
        qt_pool = outer.enter_context(tc.tile_pool(name="qtp", bufs=1))
        kt_pool = outer.enter_context(tc.tile_pool(name="ktp", bufs=1))

        identity = const.tile([P, P], f32, name="identity")
        make_identity(nc, identity)
        onescratch = const.tile([P, P], f32, name="onescratch")
        nc.vector.memset(onescratch, 1.0)
        ones128 = const.tile([P, P], f32r, name="ones128")
        nc.vector.tensor_copy(out=ones128, in_=onescratch)
        eps_t = const.tile([P, 1], f32, name="eps_t")
        nc.vector.memset(eps_t, LN_EPS)
        if has_bias:
            onesrow = const.tile([1, QC], f32r, name="onesrow")
            nc.vector.tensor_copy(out=onesrow, in_=onescratch[0:1, :QC].bitcast(f32))
            cq_sb = const.tile([1, D], f32r, name="cq_sb")
            ck_sb = const.tile([1, D], f32r, name="ck_sb")
            cv_sb = const.tile([1, D], f32r, name="cv_sb")
            nc.sync.dma_start(out=cq_sb, in_=cq[:])
            nc.sync.dma_start(out=ck_sb, in_=ck[:])
            nc.sync.dma_start(out=cv_sb, in_=cv[:])

        v_dram = dram.tile([S, D], f32r, name="v_dram")

        wo_sb = wo_pool.tile([P, DT, D], f32r, name="wo_sb")
        nc.sync.dma_start(out=wo_sb, in_=wot[:].rearrange("(o i) e -> i o e", i=P))

        QT = [qt_pool.tile([P, SQ], f32r, tag=f"qt{e}", name=f"QT{e}")
              for e in range(DT)]
        KT = [kt_pool.tile([P, S], f32r, tag=f"kt{e}", name=f"KT{e}")
              for e in range(DT)]

        # ---------------- Phase 1+2 pools (released before phase 3) --------
        with ExitStack() as ph12:
            wproj = ph12.enter_context(tc.tile_pool(name="wproj", bufs=2))
            xpool = ph12.enter_context(tc.tile_pool(name="xpool", bufs=3))
            statp = ph12.enter_context(tc.tile_pool(name="statp", bufs=4))
            ytpool = ph12.enter_context(tc.tile_pool(name="ytpool", bufs=1))
            vstage = ph12.enter_context(tc.tile_pool(name="vstage", bufs=3))
            tpsum = ph12.enter_context(
                tc.tile_pool(name="tpsum", bufs=3, space="PSUM"))
            qkvps = ph12.enter_context(
                tc.tile_pool(name="qkvps", bufs=3, space="PSUM"))

            wq_sb = wproj.tile([P, DT, D], f32r, tag="w", name="wq_sb")
            nc.sync.dma_start(out=wq_sb,
                              in_=wqt[:].rearrange("(o i) e -> i o e", i=P))

            yT = [ytpool.tile([P, S], f32r, tag=f"yt{e}", name=f"yT{e}")
                  for e in range(DT)]

            # ---- Phase 1: LayerNorm (token-major) + transpose to yT ----
            for i in range(ST):
                xt = xpool.tile([P, D], f32, tag="xt", name="xt")
                nc.sync.dma_start(out=xt, in_=x[i * P:(i + 1) * P, :])
                stats = statp.tile([P, 3, 6], f32, tag="stats", name="stats")
                for g3 in range(3):
                    nc.vector.bn_stats(out=stats[:, g3, :],
                                       in_=xt[:, g3 * 256:(g3 + 1) * 256])
                mv = statp.tile([P, 2], f32, tag="mv", name="mv")
                nc.vector.bn_aggr(out=mv, in_=stats)
                rstd = statp.tile([P, 1], f32, tag="rstd", name="rstd")
                nc.scalar.activation(out=rstd, in_=mv[:, 1:2], func=Sqrt,
                                     bias=eps_t)
                nc.vector.reciprocal(out=rstd, in_=rstd)
                # y = (x - mean) * rstd, in place
                nc.vector.tensor_scalar(out=xt, in0=xt, scalar1=mv[:, 0:1],
                                        scalar2=rstd, op0=sub, op1=mult)
                for db in range(DT):
                    pt = tpsum.tile([P, P], f32, tag="tp", name="pt")
                    nc.tensor.transpose(pt, xt[:, db * P:(db + 1) * P], identity)
                    nc.vector.tensor_copy(out=yT[db][:, i * P:(i + 1) * P],
                                          in_=pt)

            # ---- Phase 2a: QT[e, q] for own queries ----
            for eb in range(DT):
                for ch in range(SQ // QC):
                    ps = qkvps.tile([P, QC], f32, tag="qkv", name="psq")
                    for db in range(DT):
                        nc.tensor.matmul(
                            ps, mm(wq_sb[:, db, eb * P:(eb + 1) * P]),
                            mm(yT[db][:, ch * QC:(ch + 1) * QC]),
                            start=(db == 0),
                            stop=(db == DT - 1 and not has_bias))
                    if has_bias:
                        nc.tensor.matmul(ps, mm(cq_sb[0:1, eb * P:(eb + 1) * P]),
                                         mm(onesrow[0:1, :QC]),
                                         start=False, stop=True)
                    nc.vector.tensor_copy(out=QT[eb][:, ch * QC:(ch + 1) * QC],
                                          in_=ps)

            wk_sb = wproj.tile([P, DT, D], f32r, tag="w", name="wk_sb")
            nc.sync.dma_start(out=wk_sb,
                              in_=wkt[:].rearrange("(o i) e -> i o e", i=P))

            # ---- Phase 2b: KT[e, k] for all keys ----
            for eb in range(DT):
                for ch in range(S // QC):
                    ps = qkvps.tile([P, QC], f32, tag="qkv", name="psk")
                    for db in range(DT):
                        nc.tensor.matmul(
                            ps, mm(wk_sb[:, db, eb * P:(eb + 1) * P]),
                            mm(yT[db][:, ch * QC:(ch + 1) * QC]),
                            start=(db == 0),
                            stop=(db == DT - 1 and not has_bias))
                    if has_bias:
                        nc.tensor.matmul(ps, mm(ck_sb[0:1, eb * P:(eb + 1) * P]),
                                         mm(onesrow[0:1, :QC]),
                                         start=False, stop=True)
                    nc.vector.tensor_copy(out=KT[eb][:, ch * QC:(ch + 1) * QC],
                                          in_=ps)

            wv_sb = wproj.tile([P, DT, D], f32r, tag="w", name="wv_sb")
            nc.sync.dma_start(out=wv_sb,
                              in_=wvt[:].rearrange("(o i) e -> i o e", i=P))

            # ---- Phase 2c: V[k, e] token-major, spilled to DRAM ----
            EW = 384  # half of D per matmul
            for sb in range(ST):
                vs = vstage.tile([P, D], f32r, tag="vs", name="vs")
                for ch in range(D // EW):
                    ps = qkvps.tile([P, EW], f32, tag="qkv", name="psv")
                    for db in range(DT):
                        nc.tensor.matmul(
                            ps, mm(yT[db][:, sb * P:(sb + 1) * P]),
                            mm(wv_sb[:, db, ch * EW:(ch + 1) * EW]),
                            start=(db == 0),
                            stop=(db == DT - 1 and not has_bias))
                    if has_bias:
                        nc.tensor.matmul(ps, mm(ones128[0:1, :P]),
                                         mm(cv_sb[0:1, ch * EW:(ch + 1) * EW]),
                                         start=False, stop=True)
                    nc.vector.tensor_copy(out=vs[:, ch * EW:(ch + 1) * EW],
                                          in_=ps)
                nc.sync.dma_start(out=v_dram[sb * P:(sb + 1) * P, :], in_=vs)

        # ---------------- Phase 3: attention + output, per query chunk -----
        with ExitStack() as ph3:
            sb3 = ph3.enter_context(tc.tile_pool(name="sb3", bufs=1))
            psb = ph3.enter_context(tc.tile_pool(name="psb", bufs=1, space="PSUM"))

            for ch in range(NCH):
                q0 = ch * QC
                attn_ps = [psb.tile([P, QC], f32, tag=f"attn{e}", name=f"aps{e}")
                           for e in range(DT)]
                dacc = sb3.tile([P, QC], f32r, tag="dacc", bufs=2, name="dacc")
                exps = {}

                def mm2(kb, ch=ch, q0=q0, attn_ps=attn_ps, dacc=dacc, exps=exps):
                    sc = psb.tile([P, QC], f32, tag="scores", bufs=2, name="sc")
                    for et in range(DT):
                        nc.tensor.matmul(sc, mm(KT[et][:, kb * P:(kb + 1) * P]),
                                         mm(QT[et][:, q0:q0 + QC]),
                                         start=(et == 0), stop=(et == DT - 1),
                                         skip_group_check=True)
                    if use_mask:
                        mt = sb3.tile([P, QC], f32, tag="mt", bufs=4, name="mt")
                        nc.sync.dma_start(
                            out=mt, in_=amask[kb * P:(kb + 1) * P, q0:q0 + QC])
                        nc.vector.tensor_add(sc, sc, mt)
                    ex = sb3.tile([P, QC], f32r, tag="exp", bufs=4, name="ex")
                    nc.scalar.activation(out=ex, in_=sc, func=Exp)
                    if kb == 0:
                        nc.vector.tensor_copy(out=dacc, in_=ex)
                    else:
                        nc.vector.tensor_add(dacc, dacc, ex)
                    exps[kb] = ex

                def mm3(kb, attn_ps=attn_ps, exps=exps):
                    vt = sb3.tile([P, D], f32r, tag="vload", bufs=4, name="vt")
                    nc.sync.dma_start(out=vt, in_=v_dram[kb * P:(kb + 1) * P, :])
                    for e2 in range(DT):
                        nc.tensor.matmul(attn_ps[e2],
                                         mm(vt[:, e2 * P:(e2 + 1) * P]),
                                         mm(exps[kb]),
                                         start=(kb == 0), stop=(kb == KB - 1),
                                         skip_group_check=True)
                    del exps[kb]

                for kb in range(KB):
                    mm2(kb)
                    if kb >= 2:
                        mm3(kb - 2)
                mm3(KB - 2)
                mm3(KB - 1)

                # denominator: partition-reduce dacc and broadcast via ones-matmul
                dps = psb.tile([P, QC], f32, tag="scores", bufs=2, name="dps")
                nc.tensor.matmul(dps, mm(ones128), mm(dacc), start=True,
                                 stop=True, skip_group_check=True)
                recip = sb3.tile([P, QC], f32, tag="recip", bufs=2, name="recip")
                nc.vector.reciprocal(recip, dps)

                ans = []
                for e2 in range(DT):
                    an = sb3.tile([P, QC], f32r, tag=f"an{e2}", bufs=2,
                                  name=f"an{e2}")
                    nc.vector.tensor_mul(an, attn_ps[e2], recip)
                    ans.append(an)

                # MM4: out[q, f] = attn_norm @ Wo^T + residual
                for qb in range(QC // P):
                    row = q0 + qb * P
                    rt = sb3.tile([P, D], f32, tag="resid", bufs=3, name="rt")
                    nc.sync.dma_start(out=rt, in_=x[row:row + P, :])
                    ot = sb3.tile([P, D], f32, tag="outt", bufs=3, name="ot")
                    for f0, fw in ((0, 512), (512, 256)):
                        op = psb.tile([P, fw], f32, tag="scores", bufs=2,
                                      padded_shape=[P, QC], name="op")
                        for et in range(DT):
                            nc.tensor.matmul(
                                op, mm(ans[et][:, qb * P:(qb + 1) * P]),
                                mm(wo_sb[:, et, f0:f0 + fw]),
                                start=(et == 0), stop=(et == DT - 1),
                                skip_group_check=True)
                        nc.vector.tensor_add(ot[:, f0:f0 + fw], op,
                                             rt[:, f0:f0 + fw])
                    nc.sync.dma_start(out=out_d[row:row + P, :], in_=ot)

    nc.compile()
    return nc


def _get_nc(has_bias: bool, use_mask: bool, use_f32r: bool = True):
    key = (has_bias, use_mask, use_f32r)
    if key not in _BUILD_CACHE:
        _BUILD_CACHE[key] = _build(*key)
    return _BUILD_CACHE[key]


def _round_f32r(a):
    """Round fp32 to the fp32r (e8m11) grid, round-to-nearest-even."""
    bits = np.ascontiguousarray(a, np.float32).view(np.uint32)
    keep = np.uint32(0xFFFFF000)
    lsb = (bits >> np.uint32(12)) & np.uint32(1)
    rounded = (bits + np.uint32(0x7FF) + lsb) & keep
    return rounded.view(np.float32)


def kernel(x, mask, Wq, Wk, Wv, Wo, ln_g, ln_b):
    from concourse.bass_utils import run_bass_kernel_spmd

    x = np.asarray(x, np.float32)
    mask = np.asarray(mask)
    ln_g = np.asarray(ln_g, np.float32)
    ln_b = np.asarray(ln_b, np.float32)
    has_bias = bool(np.any(ln_b != 0.0))
    use_mask = not bool(np.all(mask == 1))

    nc = _get_nc(has_bias, use_mask)

    scale = np.float32(1.0 / np.sqrt(D))
    wq_f = np.asarray(Wq, np.float32) * ln_g[None, :]
    wk_f = np.asarray(Wk, np.float32) * ln_g[None, :]
    wv_f = np.asarray(Wv, np.float32) * ln_g[None, :]
    wqt = _round_f32r(np.ascontiguousarray(wq_f.T * scale, np.float32))
    wkt = _round_f32r(np.ascontiguousarray(wk_f.T, np.float32))
    wvt = _round_f32r(np.ascontiguousarray(wv_f.T, np.float32))
    wot = _round_f32r(np.ascontiguousarray(np.asarray(Wo, np.float32).T,
                                           np.float32))

    in_maps = []
    for c in range(N_CORES):
        b, qh = divmod(c, 2)
        qsl = slice(qh * SQ, (qh + 1) * SQ)
        osl = slice((1 - qh) * SQ, (2 - qh) * SQ)
        xa = np.ascontiguousarray(
            np.concatenate([x[b, qsl], x[b, osl]], axis=0), np.float32)
        m = {"x": xa, "wqt": wqt, "wkt": wkt, "wvt": wvt, "wot": wot}
        if has_bias:
            m["cq"] = _round_f32r(np.ascontiguousarray(
                (wq_f @ ln_b)[None, :] * scale, np.float32))
            m["ck"] = _round_f32r(
                np.ascontiguousarray((wk_f @ ln_b)[None, :], np.float32))
            m["cv"] = _round_f32r(
                np.ascontiguousarray((wv_f @ ln_b)[None, :], np.float32))
        if use_mask:
            # additive mask, [k_arranged, q_own]
            kmat = np.concatenate([mask[b][qsl][:, qsl], mask[b][qsl][:, osl]],
                                  axis=1)  # [q_own, k_arranged]
            m["amask"] = np.ascontiguousarray(
                ((1.0 - kmat.T) * np.float32(-1e9)), np.float32)
        in_maps.append(m)

    res = run_bass_kernel_spmd(nc, in_maps, core_ids=list(range(N_CORES)))

    out = np.empty((B, S, D), np.float32)
    for c in range(N_CORES):
        b, qh = divmod(c, 2)
        out[b, qh * SQ:(qh + 1) * SQ] = res.results[c]["out"]
    return out


# revision 19
# speedup vs baseline: 351.4997x; 351.4997x over previous
"""Self-contained Trainium2 Bass kernel for CoherenceAttention.

Problem: out = x + Softmax(mask, (LN(x) Wq^T)(LN(x) Wk^T)^T / sqrt(D)) (LN(x) Wv^T) Wo^T
Shapes: x (4, 2048, 768), weights (768, 768), LN affine (768,).

Sharding: 8 cores = (batch, query-half). Each core receives its batch's x with
its own 1024 query rows first (attention is permutation-invariant over keys),
computes LN + K/V for all 2048 keys (duplicated within the batch pair; no
collectives), and Q/scores/softmax/output projection for its 1024 queries.

Device kernel layout strategy (per core):
  P1: token-major LN (bn_stats/bn_aggr) then PE-transpose -> yT [d, s] (feature-major)
  P2: QT = (Wq' yT)  [e, q] ; KT [e, k] ; V token-major [k, e] spilled to DRAM
  P3: per 512-query chunk: scoresT[k,q] in PSUM -> exp on ACT -> SBUF;
      attn_outT[e,q] += V-block^T-free matmuls over k; denominator via
      elementwise-accumulate + ones-matmul partition reduce (broadcast form);
      normalize; out[q,f] = attn_norm^T-matmuls with Wo^T + residual.
All matmuls run as float32r (full fp32 data, 1 cycle/row on the PE).
"""

import numpy as np

B, S, D = 4, 2048, 768
N_CORES = 8
P = 128
SQ = S // 2           # queries per core
DT = D // P           # 6 contraction tiles
ST = S // P           # 16 token tiles
KB = S // P           # 16 key blocks
QC = 512              # query chunk (PSUM bank width in fp32)
NCH = SQ // QC        # 2 chunks per core
LN_EPS = 1e-5
VKEEP = 10            # V key-blocks kept resident in SBUF (rest spilled)

_BUILD_CACHE = {}


def _build(has_bias: bool, use_mask: bool, use_f32r: bool):
    import concourse.bacc as bacc
    import concourse.mybir as mybir
    import concourse.tile as tile
    from concourse.masks import make_identity
    from contextlib import ExitStack

    f32 = mybir.dt.float32
    f32r = mybir.dt.float32r if use_f32r else f32

    def mm(ap):
        return ap

    nc = bacc.Bacc("TRN2", target_bir_lowering=False, debug=False,
                   num_devices=N_CORES)

    x = nc.dram_tensor("x", [S, D], f32, kind="ExternalInput")
    wqt = nc.dram_tensor("wqt", [D, D], f32r, kind="ExternalInput")
    wkt = nc.dram_tensor("wkt", [D, D], f32r, kind="ExternalInput")
    wvt = nc.dram_tensor("wvt", [D, D], f32r, kind="ExternalInput")
    wot = nc.dram_tensor("wot", [D, D], f32r, kind="ExternalInput")
    if has_bias:
        cq = nc.dram_tensor("cq", [1, D], f32r, kind="ExternalInput")
        ck = nc.dram_tensor("ck", [1, D], f32r, kind="ExternalInput")
        cv = nc.dram_tensor("cv", [1, D], f32r, kind="ExternalInput")
    if use_mask:
        amask = nc.dram_tensor("amask", [S, SQ], f32, kind="ExternalInput")
    out_d = nc.dram_tensor("out", [SQ, D], f32, kind="ExternalOutput")

    sub = mybir.AluOpType.subtract
    mult = mybir.AluOpType.mult
    Exp = mybir.ActivationFunctionType.Exp
    Sqrt = mybir.ActivationFunctionType.Sqrt

    with tile.TileContext(nc) as tc, ExitStack() as outer:
        const = outer.enter_context(tc.tile_pool(name="const", bufs=1))
        dram = outer.enter_context(tc.tile_pool(name="dram", bufs=1, space="DRAM"))
        qt_pool = outer.enter_context(tc.tile_pool(name="qtp", bufs=1))
        kt_pool = outer.enter_context(tc.tile_pool(name="ktp", bufs=1))
        vk_pool = outer.enter_context(tc.tile_pool(name="vkp", bufs=1))

        onescratch = const.tile([P, P], f32, name="onescratch")
        nc.vector.memset(onescratch, 0.0)
        make_identity(nc, onescratch, nomemset=True)
        identity = const.tile([P, P], f32r, name="identity")
        nc.vector.tensor_copy(out=identity, in_=onescratch)
        nc.vector.memset(onescratch, 1.0)
        ones128 = const.tile([P, P], f32r, name="ones128")
        nc.vector.tensor_copy(out=ones128, in_=onescratch)
        identity_r = identity
        eps_t = const.tile([P, 1], f32, name="eps_t")
        nc.vector.memset(eps_t, LN_EPS)
        if has_bias:
            onesrow = const.tile([1, QC], f32r, name="onesrow")
            nc.vector.tensor_copy(out=onesrow, in_=onescratch[0:1, :QC].bitcast(f32))
            cq_sb = const.tile([1, D], f32r, name="cq_sb")
            ck_sb = const.tile([1, D], f32r, name="ck_sb")
            cv_sb = const.tile([1, D], f32r, name="cv_sb")
            nc.sync.dma_start(out=cq_sb, in_=cq[:])
            nc.sync.dma_start(out=ck_sb, in_=ck[:])
            nc.sync.dma_start(out=cv_sb, in_=cv[:])

        v_dram = dram.tile([(ST - VKEEP) * P, D], f32r, name="v_dram")


        QT = [qt_pool.tile([P, SQ], f32r, tag=f"qt{e}", name=f"QT{e}")
              for e in range(DT)]
        vkeep_tiles = [vk_pool.tile([P, D], f32r, tag=f"vk{i}", name=f"vk{i}")
                       for i in range(VKEEP)]
        KT = [kt_pool.tile([P, S], f32r, tag=f"kt{e}", name=f"KT{e}")
              for e in range(DT)]

        # ---------------- Phase 1+2 pools (released before phase 3) --------
        with ExitStack() as ph12:
            wproj = ph12.enter_context(tc.tile_pool(name="wproj", bufs=2))
            xpool = ph12.enter_context(tc.tile_pool(name="xpool", bufs=2))
            ypool = ph12.enter_context(tc.tile_pool(name="ypool", bufs=2))
            statp = ph12.enter_context(tc.tile_pool(name="statp", bufs=4))
            ytpool = ph12.enter_context(tc.tile_pool(name="ytpool", bufs=1))
            vstage = ph12.enter_context(tc.tile_pool(name="vstage", bufs=2))
            tpsum = ph12.enter_context(
                tc.tile_pool(name="tpsum", bufs=3, space="PSUM"))
            qkvps = ph12.enter_context(
                tc.tile_pool(name="qkvps", bufs=3, space="PSUM"))

            wq_sb = wproj.tile([P, DT, D], f32r, tag="w", name="wq_sb")
            wq_sb_src = wqt[:].rearrange("(o i) e -> i o e", i=P)

            def load_wq():
                for _wc in range(3):
                    nc.sync.dma_start(
                        out=wq_sb[:, 2 * _wc:2 * _wc + 2, :],
                        in_=wq_sb_src[:, 2 * _wc:2 * _wc + 2, :])

            yT = [ytpool.tile([P, S], f32r, tag=f"yt{e}", name=f"yT{e}")
                  for e in range(DT)]

            # ---- Phase 1: LayerNorm (token-major) + transpose to yT.
            # Split in halves: query tiles (0-7) first, then QT runs while
            # the second half of x still streams in. ----
            def ln_tile(i):
                xt = xpool.tile([P, D], f32, tag="xt", name="xt")
                nc.sync.dma_start(out=xt, in_=x[i * P:(i + 1) * P, :])
                stats = statp.tile([P, 3, 6], f32, tag="stats", name="stats")
                for g3 in range(3):
                    nc.vector.bn_stats(out=stats[:, g3, :],
                                       in_=xt[:, g3 * 256:(g3 + 1) * 256])
                mv = statp.tile([P, 2], f32, tag="mv", name="mv")
                nc.vector.bn_aggr(out=mv, in_=stats)
                rstd = statp.tile([P, 1], f32, tag="rstd", name="rstd")
                nc.scalar.activation(out=rstd, in_=mv[:, 1:2], func=Sqrt,
                                     bias=eps_t)
                nc.vector.reciprocal(out=rstd, in_=rstd)
                # y = (x - mean) * rstd -> separate f32r tile (rounded)
                xtr = ypool.tile([P, D], f32r, tag="yt", name="ytile")
                nc.vector.tensor_scalar(out=xtr, in0=xt,
                                        scalar1=mv[:, 0:1],
                                        scalar2=rstd, op0=sub, op1=mult)
                for db in range(DT):
                    pt = tpsum.tile([P, P], f32r, tag="tp", name="pt")
                    nc.tensor.transpose(pt, xtr[:, db * P:(db + 1) * P],
                                        identity_r)
                    nc.scalar.copy(out=yT[db][:, i * P:(i + 1) * P], in_=pt)

            for i in range(ST):
                ln_tile(i)
                if i == 1:
                    load_wq()


            # ---- Phase 2a: QT[e, q] for own queries (runs while the second
            # half of x still streams) ----
            for eb in range(DT):
                for ch in range(SQ // QC):
                    ps = qkvps.tile([P, QC], f32, tag="qkv", name="psq")
                    for db in range(DT):
                        nc.tensor.matmul(
                            ps, mm(wq_sb[:, db, eb * P:(eb + 1) * P]),
                            mm(yT[db][:, ch * QC:(ch + 1) * QC]),
                            start=(db == 0),
                            stop=(db == DT - 1 and not has_bias))
                    if has_bias:
                        nc.tensor.matmul(ps, mm(cq_sb[0:1, eb * P:(eb + 1) * P]),
                                         mm(onesrow[0:1, :QC]),
                                         start=False, stop=True)
                    nc.vector.tensor_copy(out=QT[eb][:, ch * QC:(ch + 1) * QC],
                                          in_=ps)
            wk_sb = wproj.tile([P, DT, D], f32r, tag="w", name="wk_sb")
            wk_sb_src = wkt[:].rearrange("(o i) e -> i o e", i=P)
            for _wc in range(3):
                nc.sync.dma_start(
                    out=wk_sb[:, 2 * _wc:2 * _wc + 2, :],
                    in_=wk_sb_src[:, 2 * _wc:2 * _wc + 2, :])

            # ---- Phase 2b: KT[e, k] for all keys ----
            for eb in range(DT):
                for ch in range(S // QC):
                    ps = qkvps.tile([P, QC], f32, tag="qkv", name="psk")
                    for db in range(DT):
                        nc.tensor.matmul(
                            ps, mm(wk_sb[:, db, eb * P:(eb + 1) * P]),
                            mm(yT[db][:, ch * QC:(ch + 1) * QC]),
                            start=(db == 0),
                            stop=(db == DT - 1 and not has_bias))
                    if has_bias:
                        nc.tensor.matmul(ps, mm(ck_sb[0:1, eb * P:(eb + 1) * P]),
                                         mm(onesrow[0:1, :QC]),
                                         start=False, stop=True)
                    nc.vector.tensor_copy(out=KT[eb][:, ch * QC:(ch + 1) * QC],
                                          in_=ps)

            wv_sb = wproj.tile([P, DT, D], f32r, tag="w", name="wv_sb")
            wv_sb_src = wvt[:].rearrange("(o i) e -> i o e", i=P)
            for _wc in range(3):
                nc.sync.dma_start(
                    out=wv_sb[:, 2 * _wc:2 * _wc + 2, :],
                    in_=wv_sb_src[:, 2 * _wc:2 * _wc + 2, :])

            # ---- Phase 2c: V[k, e] token-major; keep VKEEP blocks in
            # SBUF, spill the rest to DRAM ----
            EW = 384  # half of D per matmul
            for sb in range(ST):
                if sb < VKEEP:
                    vs = vkeep_tiles[sb]
                else:
                    vs = vstage.tile([P, D], f32r, tag="vs", name="vs")
                for ch in range(D // EW):
                    ps = qkvps.tile([P, EW], f32, tag="qkv", name="psv")
                    for db in range(DT):
                        nc.tensor.matmul(
                            ps, mm(yT[db][:, sb * P:(sb + 1) * P]),
                            mm(wv_sb[:, db, ch * EW:(ch + 1) * EW]),
                            start=(db == 0),
                            stop=(db == DT - 1 and not has_bias))
                    if has_bias:
                        nc.tensor.matmul(ps, mm(ones128[0:1, :P]),
                                         mm(cv_sb[0:1, ch * EW:(ch + 1) * EW]),
                                         start=False, stop=True)
                    nc.vector.tensor_copy(out=vs[:, ch * EW:(ch + 1) * EW],
                                          in_=ps)
                if sb >= VKEEP:
                    nc.sync.dma_start(
                        out=v_dram[(sb - VKEEP) * P:(sb - VKEEP + 1) * P, :],
                        in_=vs)

        # ---------------- Phase 3: attention + output, per query chunk -----
        with ExitStack() as ph3:
            sb3 = ph3.enter_context(tc.tile_pool(name="sb3", bufs=1))
            wo_pool = ph3.enter_context(tc.tile_pool(name="wop", bufs=1))
            wo_sb = wo_pool.tile([P, DT, D], f32r, name="wo_sb")
            wo_src = wot[:].rearrange("(o i) e -> i o e", i=P)
            for _wc in range(3):
                nc.sync.dma_start(out=wo_sb[:, 2 * _wc:2 * _wc + 2, :],
                                    in_=wo_src[:, 2 * _wc:2 * _wc + 2, :])
            vspill_tiles = [sb3.tile([P, D], f32r, tag=f"vsp{i}",
                                     name=f"vsp{i}")
                            for i in range(ST - VKEEP)]
            psb = ph3.enter_context(tc.tile_pool(name="psb", bufs=1, space="PSUM"))

            chunk_attn = {}
            chunk_ans = {}

            def p3_scores(ch):
                q0 = ch * QC
                attn_ps = [psb.tile([P, QC], f32, tag=f"attn{e}",
                                    name=f"aps{e}") for e in range(DT)]
                dacc = sb3.tile([P, QC], f32r, tag="dacc", bufs=2, name="dacc")
                exps = {}

                def mm2(kb):
                    sc = psb.tile([P, QC], f32, tag="scores", bufs=2, name="sc")
                    for et in range(DT):
                        nc.tensor.matmul(sc, mm(KT[et][:, kb * P:(kb + 1) * P]),
                                         mm(QT[et][:, q0:q0 + QC]),
                                         start=(et == 0), stop=(et == DT - 1),
                                         skip_group_check=True)
                    if use_mask:
                        mt = sb3.tile([P, QC], f32, tag="mt", bufs=4, name="mt")
                        nc.sync.dma_start(
                            out=mt, in_=amask[kb * P:(kb + 1) * P, q0:q0 + QC])
                        nc.vector.tensor_add(sc, sc, mt)
                    ex = sb3.tile([P, QC], f32r, tag="exp", bufs=4, name="ex")
                    nc.scalar.activation(out=ex, in_=sc, func=Exp)
                    if kb == 0:
                        nc.vector.tensor_copy(out=dacc, in_=ex)
                    else:
                        nc.vector.tensor_add(dacc, dacc, ex)
                    exps[kb] = ex

                def mm3(kb):
                    if kb < VKEEP:
                        vt = vkeep_tiles[kb]
                    elif ch == 0:
                        vt = vspill_tiles[kb - VKEEP]
                        nc.sync.dma_start(
                            out=vt, in_=v_dram[(kb - VKEEP) * P:
                                               (kb - VKEEP + 1) * P, :])
                    else:
                        vt = vspill_tiles[kb - VKEEP]
                    for e2 in range(DT):
                        nc.tensor.matmul(attn_ps[e2],
                                         mm(vt[:, e2 * P:(e2 + 1) * P]),
                                         mm(exps[kb]),
                                         start=(kb == 0), stop=(kb == KB - 1),
                                         skip_group_check=True)
                    del exps[kb]

                for kb in range(KB):
                    mm2(kb)
                    if kb >= 2:
                        mm3(kb - 2)
                mm3(KB - 2)
                mm3(KB - 1)

                # denominator: partition-reduce dacc, broadcast via ones-matmul
                dps = psb.tile([P, QC], f32, tag="scores", bufs=2, name="dps")
                nc.tensor.matmul(dps, mm(ones128), mm(dacc), start=True,
                                 stop=True, skip_group_check=True)
                chunk_attn[ch] = (attn_ps, dps)

            def p3_norm(ch):
                attn_ps, dps = chunk_attn[ch]
                recip = sb3.tile([P, QC], f32, tag="recip", bufs=2,
                                 name="recip")
                nc.vector.reciprocal(recip, dps)
                ans = []
                for e2 in range(DT):
                    an = sb3.tile([P, QC], f32r, tag=f"an{e2}", bufs=2,
                                  name=f"an{e2}")
                    nc.vector.tensor_mul(an, attn_ps[e2], recip)
                    ans.append(an)
                chunk_ans[ch] = ans

            def p3_out(ch):
                q0 = ch * QC
                ans = chunk_ans[ch]
                for qb in range(QC // P):
                    row = q0 + qb * P
                    rt = sb3.tile([P, D], f32, tag="resid", bufs=3, name="rt")
                    nc.sync.dma_start(out=rt, in_=x[row:row + P, :])
                    ot = sb3.tile([P, D], f32, tag="outt", bufs=3, name="ot")
                    for f0, fw in ((0, 512), (512, 256)):
                        op = psb.tile([P, fw], f32, tag="scores", bufs=2,
                                      padded_shape=[P, QC], name="op")
                        for et in range(DT):
                            nc.tensor.matmul(
                                op, mm(ans[et][:, qb * P:(qb + 1) * P]),
                                mm(wo_sb[:, et, f0:f0 + fw]),
                                start=(et == 0), stop=(et == DT - 1),
                                skip_group_check=True)
                        nc.vector.tensor_add(ot[:, f0:f0 + fw], op,
                                             rt[:, f0:f0 + fw])
                    nc.sync.dma_start(out=out_d[row:row + P, :], in_=ot)

            p3_scores(0)
            p3_norm(0)
            p3_scores(1)
            p3_norm(1)
            p3_out(0)
            p3_out(1)

    nc.compile()
    return nc


def _get_nc(has_bias: bool, use_mask: bool, use_f32r: bool = True):
    key = (has_bias, use_mask, use_f32r)
    if key not in _BUILD_CACHE:
        _BUILD_CACHE[key] = _build(*key)
    return _BUILD_CACHE[key]


def _round_f32r(a):
    """Round fp32 to the fp32r (e8m11) grid, round-to-nearest-even."""
    bits = np.ascontiguousarray(a, np.float32).view(np.uint32)
    keep = np.uint32(0xFFFFF000)
    lsb = (bits >> np.uint32(12)) & np.uint32(1)
    rounded = (bits + np.uint32(0x7FF) + lsb) & keep
    return rounded.view(np.float32)


def kernel(x, mask, Wq, Wk, Wv, Wo, ln_g, ln_b):
    from concourse.bass_utils import run_bass_kernel_spmd

    x = np.asarray(x, np.float32)
    mask = np.asarray(mask)
    ln_g = np.asarray(ln_g, np.float32)
    ln_b = np.asarray(ln_b, np.float32)
    has_bias = bool(np.any(ln_b != 0.0))
    use_mask = not bool(np.all(mask == 1))

    nc = _get_nc(has_bias, use_mask)

    scale = np.float32(1.0 / np.sqrt(D))
    wq_f = np.asarray(Wq, np.float32) * ln_g[None, :]
    wk_f = np.asarray(Wk, np.float32) * ln_g[None, :]
    wv_f = np.asarray(Wv, np.float32) * ln_g[None, :]
    wqt = _round_f32r(np.ascontiguousarray(wq_f.T * scale, np.float32))
    wkt = _round_f32r(np.ascontiguousarray(wk_f.T, np.float32))
    wvt = _round_f32r(np.ascontiguousarray(wv_f.T, np.float32))
    wot = _round_f32r(np.ascontiguousarray(np.asarray(Wo, np.float32).T,
                                           np.float32))

    in_maps = []
    for c in range(N_CORES):
        b, qh = divmod(c, 2)
        qsl = slice(qh * SQ, (qh + 1) * SQ)
        osl = slice((1 - qh) * SQ, (2 - qh) * SQ)
        xa = np.ascontiguousarray(
            np.concatenate([x[b, qsl], x[b, osl]], axis=0), np.float32)
        m = {"x": xa, "wqt": wqt, "wkt": wkt, "wvt": wvt, "wot": wot}
        if has_bias:
            m["cq"] = _round_f32r(np.ascontiguousarray(
                (wq_f @ ln_b)[None, :] * scale, np.float32))
            m["ck"] = _round_f32r(
                np.ascontiguousarray((wk_f @ ln_b)[None, :], np.float32))
            m["cv"] = _round_f32r(
                np.ascontiguousarray((wv_f @ ln_b)[None, :], np.float32))
        if use_mask:
            # additive mask, [k_arranged, q_own]
            kmat = np.concatenate([mask[b][qsl][:, qsl], mask[b][qsl][:, osl]],
                                  axis=1)  # [q_own, k_arranged]
            m["amask"] = np.ascontiguousarray(
                ((1.0 - kmat.T) * np.float32(-1e9)), np.float32)
        in_maps.append(m)

    res = run_bass_kernel_spmd(nc, in_maps, core_ids=list(range(N_CORES)))

    out = np.empty((B, S, D), np.float32)
    for c in range(N_CORES):
        b, qh = divmod(c, 2)
        out[b, qh * SQ:(qh + 1) * SQ] = res.results[c]["out"]
    return out
